# revision 21
# baseline (speedup 1.0000x reference)
"""Trainium2 Bass kernel for nn_ConvParaLIF: Conv2d(128,128,3x3,pad=1) followed
by FFT-based parallel leaky integration over the channel/time axis.

Key identity: the FFT stage is multiplication by a constant T x T matrix
    G[t, u] = g[(t - u) mod 2T],  g = conv(alpha^t, beta^t * (1 - beta))
(zero-padded length-2T circular convolution, exactly as the reference computes
it). Both the conv and the time-mix are linear in the channel axis, so G is
fused into the conv weights on device with 9 small 128x128x128 fp32 matmuls:
    Wf_k[t, c] = sum_u G[t, u] W[u, c, k]
after which the whole problem is a single 3x3 conv, executed as 9 accumulated
matmuls (fp16 operands, fp32 PSUM accumulate) per 4-row output chunk.

The image is zero-padded (and cast) host-side so the per-image input DMA is a
single fully-contiguous transfer and no on-device border memsets are needed.

Sharding: data-parallel over batch, 2 images per core on 8 cores; weights
replicated.
"""

import sys

if "/opt/trn_rl_repo" not in sys.path:
    sys.path.insert(0, "/opt/trn_rl_repo")

import numpy as np


B, C, H, W_SP = 16, 128, 112, 112
T = C  # channel axis doubles as the LIF time axis
KH = KW = 3
N_CORES = 8
IPC = B // N_CORES  # images per core
HP, WP = H + 2, W_SP + 2  # padded image
ROWS = 4  # output rows per PSUM chunk (4*112 = 448 <= 512 fp32 / bank)
GROUP = 4  # PSUM chunks per output DMA batch (16 rows / 7 KB per partition)

MM_DTYPE = "float16"  # conv matmul operand dtype ("float16" | "float32r")

_BUILD_CACHE: dict = {}


def _np_mm_dtype(mm_dtype: str):
    return np.float16 if mm_dtype == "float16" else np.float32


def _g_matrix() -> np.ndarray:
    """G[t, u] such that mem[t] = sum_u G[t, u] X[u] reproduces the reference's
    rfft(n=2T) based 'parallel leaky integration' (including its wraparound
    terms). Computed in float64, cast to fp32."""
    alpha = beta = np.exp(-1.0)
    t = np.arange(T, dtype=np.float64)
    l = alpha**t
    k = (beta**t) * (1.0 - beta)
    g = np.convolve(l, k)  # length 2T-1
    gpad = np.zeros(2 * T)
    gpad[: 2 * T - 1] = g
    G = gpad[(np.arange(T)[:, None] - np.arange(T)[None, :]) % (2 * T)]
    return G.astype(np.float32)


def _build(repeat: int = 1, mm_dtype: str = MM_DTYPE):
    """Build + compile the per-core Bass program. Returns the Bacc module."""
    key = (repeat, mm_dtype)
    if key in _BUILD_CACHE:
        return _BUILD_CACHE[key]

    import concourse.tile as tile
    from concourse import bacc, mybir

    f32 = mybir.dt.float32
    mm_dt = getattr(mybir.dt, mm_dtype)

    nc = bacc.Bacc("TRN2", target_bir_lowering=False, debug=False)
    # x arrives host-padded (HP x WP, zeros on the border) and pre-cast to the
    # matmul dtype, so each image loads as one contiguous DMA.
    x_in = nc.dram_tensor("x", [IPC, C, HP, WP], mm_dt, kind="ExternalInput")
    w_in = nc.dram_tensor("w", [C, C, KH * KW], f32, kind="ExternalInput")
    gt_in = nc.dram_tensor("gt", [T, T], f32, kind="ExternalInput")  # GT[u,t]=G[t,u]
    b_in = nc.dram_tensor("b", [C, 1], f32, kind="ExternalInput")
    y_out = nc.dram_tensor("y", [IPC, C, H, W_SP], f32, kind="ExternalOutput")

    with tile.TileContext(nc) as tc:
        with tc.tile_pool(name="consts", bufs=1) as cpool:
            w_sb = cpool.tile([C, C, KH * KW], f32)
            gt_sb = cpool.tile([T, T], f32)
            b_sb = cpool.tile([C, 1], f32)
            wk_sb = cpool.tile([C, KH * KW, T], mm_dt)  # fused conv weights (lhsT)
            bias_sb = cpool.tile([T, 1], f32)  # G @ b
            nc.sync.dma_start(w_sb[:], w_in[:])
            nc.sync.dma_start(gt_sb[:], gt_in[:])
            nc.sync.dma_start(b_sb[:], b_in[:])

            # Fuse the time-mix into the conv weights:
            #   wk_sb[c, k, t] = sum_u W[u, c, k] * G[t, u]
            # which is exactly the lhsT the conv matmuls need.
            with tc.tile_pool(name="fpsum", bufs=2, space="PSUM") as fpsum:
                for kk in range(KH * KW):
                    ps = fpsum.tile([C, T], f32)
                    nc.tensor.matmul(
                        ps[:], w_sb[:, :, kk], gt_sb[:], start=True, stop=True
                    )
                    nc.vector.tensor_copy(wk_sb[:, kk, :], ps[:])
                psb = fpsum.tile([T, 1], f32, tag="psb")
                nc.tensor.matmul(psb[:], gt_sb[:], b_sb[:], start=True, stop=True)
                nc.vector.tensor_copy(bias_sb[:], psb[:])

            with (
                tc.tile_pool(name="pad", bufs=2) as pad_pool,
                tc.tile_pool(name="outs", bufs=3) as out_pool,
                tc.tile_pool(name="cpsum", bufs=6, space="PSUM") as cpsum,
            ):
                grows = GROUP * ROWS  # 16 output rows per DMA batch

                def conv_body(_iv=None):
                    for i in range(IPC):
                        pb = pad_pool.tile([C, HP, WP], mm_dt, tag="pad")
                        nc.sync.dma_start(pb[:], x_in[i])
                        for g0 in range(0, H, grows):
                            ot = out_pool.tile([C, grows, W_SP], f32, tag="ot")
                            for c in range(GROUP):
                                hh = g0 + c * ROWS
                                ps = cpsum.tile(
                                    [C, ROWS, W_SP], f32, name="cps", tag="cps"
                                )
                                for kk in range(KH * KW):
                                    dh, dw = divmod(kk, KW)
                                    rhs = pb[
                                        :, hh + dh : hh + dh + ROWS, dw : dw + W_SP
                                    ]
                                    nc.tensor.matmul(
                                        ps[:],
                                        wk_sb[:, kk, :],
                                        rhs,
                                        start=(kk == 0),
                                        stop=(kk == KH * KW - 1),
                                    )
                                nc.vector.tensor_scalar_add(
                                    ot[:, c * ROWS : (c + 1) * ROWS, :],
                                    ps[:],
                                    bias_sb[:],
                                )
                            nc.sync.dma_start(
                                y_out[i, :, g0 : g0 + grows, :], ot[:]
                            )

                if repeat == 1:
                    conv_body()
                else:
                    with tc.For_i(0, repeat, 1) as iv:
                        conv_body(iv)

    nc.compile()
    _BUILD_CACHE[key] = nc
    return nc


def _prep_inputs(x: np.ndarray, W: np.ndarray, b: np.ndarray, mm_dtype: str):
    G = _g_matrix()
    gt = np.ascontiguousarray(G.T)
    w3 = np.ascontiguousarray(W.reshape(C, C, KH * KW).astype(np.float32))
    b2 = np.ascontiguousarray(b.reshape(C, 1).astype(np.float32))
    np_dt = _np_mm_dtype(mm_dtype)
    xp = np.zeros((x.shape[0], C, HP, WP), np_dt)
    xp[:, :, 1 : H + 1, 1 : W_SP + 1] = x  # cast + zero-pad on host
    return xp, w3, gt, b2


def _run(nc, x, W, b, mm_dtype: str = MM_DTYPE, trace: bool = False):
    from concourse.bass_utils import run_bass_kernel_spmd

    xp, w3, gt, b2 = _prep_inputs(x, W, b, mm_dtype)
    core_ids = list(range(N_CORES))
    in_maps = [
        {"x": xp[c * IPC : (c + 1) * IPC], "w": w3, "gt": gt, "b": b2}
        for c in core_ids
    ]
    res = run_bass_kernel_spmd(nc, in_maps, core_ids, trace=trace)
    out = np.concatenate([res.results[c]["y"] for c in core_ids], axis=0)
    return out.reshape(B, C, H, W_SP), res


def kernel(x: np.ndarray, W: np.ndarray, b: np.ndarray) -> np.ndarray:
    nc = _build(repeat=1)
    out, _ = _run(nc, x, W, b)
    return out


if __name__ == "__main__":
    # CoreSim structural/numerical self-check on one core's worth of data.
    from concourse.bass_interp import CoreSim

    mm_dtype = sys.argv[1] if len(sys.argv) > 1 else MM_DTYPE
    rng = np.random.default_rng(0)
    x = rng.standard_normal((IPC, C, H, W_SP)).astype(np.float32)
    W = (rng.standard_normal((C, C, KH, KW)) * 0.05).astype(np.float32)
    b = np.zeros((C,), np.float32)
    G = _g_matrix()

    nc = _build(repeat=1, mm_dtype=mm_dtype)
    xp, w3, gt, b2 = _prep_inputs(x, W, b, mm_dtype)
    sim = CoreSim(nc)
    sim.tensor("x")[:] = xp[:IPC]
    sim.tensor("w")[:] = w3
    sim.tensor("gt")[:] = gt
    sim.tensor("b")[:] = b2
    sim.simulate()
    got = np.asarray(sim.tensor("y"))

    # numpy reference: conv then G-mix, in float64
    xpad = np.pad(x.astype(np.float64), ((0, 0), (0, 0), (1, 1), (1, 1)))
    conv = np.zeros((IPC, C, H, W_SP))
    for dh in range(KH):
        for dw in range(KW):
            patch = xpad[:, :, dh : dh + H, dw : dw + W_SP]
            conv += np.einsum(
                "oc,bchw->bohw", W[:, :, dh, dw].astype(np.float64), patch
            )
    want = np.einsum("tu,buhw->bthw", G.astype(np.float64), conv)
    err = np.abs(got - want).max()
    scale = np.abs(want).max()
    print(f"CoreSim max abs err: {err:.3e}  (scale {scale:.3f}, rel {err / scale:.3e})")


# revision 23
# speedup vs baseline: 1.1256x; 1.1256x over previous
"""Trainium2 Bass kernel for nn_ConvParaLIF: Conv2d(128,128,3x3,pad=1) followed
by FFT-based parallel leaky integration over the channel/time axis.

Key identity: the FFT stage is multiplication by a constant T x T matrix
    G[t, u] = g[(t - u) mod 2T],  g = conv(alpha^t, beta^t * (1 - beta))
(zero-padded length-2T circular convolution, exactly as the reference computes
it). Both the conv and the time-mix are linear in the channel axis, so G is
fused into the conv weights on device with 9 small 128x128x128 fp32 matmuls:
    Wf_k[t, c] = sum_u G[t, u] W[u, c, k]
after which the whole problem is a single 3x3 conv, executed as 9 accumulated
matmuls (fp16 operands, fp32 PSUM accumulate) per 4-row output chunk.

The image is zero-padded (and cast) host-side so the per-image input DMA is a
single fully-contiguous transfer and no on-device border memsets are needed.

Sharding: data-parallel over batch, 2 images per core on 8 cores; weights
replicated.
"""

import sys

if "/opt/trn_rl_repo" not in sys.path:
    sys.path.insert(0, "/opt/trn_rl_repo")

import numpy as np


B, C, H, W_SP = 16, 128, 112, 112
T = C  # channel axis doubles as the LIF time axis
KH = KW = 3
N_CORES = 8
IPC = B // N_CORES  # images per core
HP, WP = H + 2, W_SP + 2  # padded image
ROWS = 4  # output rows per PSUM chunk (4*112 = 448 <= 512 fp32 / bank)
GROUP = 7  # PSUM chunks per output DMA batch (28 rows / 12.5 KB per partition)
DMA_SPLIT = 60  # padded-row boundary: image loads as two DMAs so compute
# on the top half starts while the bottom half is still streaming in

MM_DTYPE = "float16"  # conv matmul operand dtype ("float16" | "float32r")

_BUILD_CACHE: dict = {}


def _np_mm_dtype(mm_dtype: str):
    return np.float16 if mm_dtype == "float16" else np.float32


def _g_matrix() -> np.ndarray:
    """G[t, u] such that mem[t] = sum_u G[t, u] X[u] reproduces the reference's
    rfft(n=2T) based 'parallel leaky integration' (including its wraparound
    terms). Computed in float64, cast to fp32."""
    alpha = beta = np.exp(-1.0)
    t = np.arange(T, dtype=np.float64)
    l = alpha**t
    k = (beta**t) * (1.0 - beta)
    g = np.convolve(l, k)  # length 2T-1
    gpad = np.zeros(2 * T)
    gpad[: 2 * T - 1] = g
    G = gpad[(np.arange(T)[:, None] - np.arange(T)[None, :]) % (2 * T)]
    return G.astype(np.float32)


def _build(repeat: int = 1, mm_dtype: str = MM_DTYPE):
    """Build + compile the per-core Bass program. Returns the Bacc module."""
    key = (repeat, mm_dtype)
    if key in _BUILD_CACHE:
        return _BUILD_CACHE[key]

    import concourse.tile as tile
    from concourse import bacc, mybir

    f32 = mybir.dt.float32
    mm_dt = getattr(mybir.dt, mm_dtype)

    nc = bacc.Bacc("TRN2", target_bir_lowering=False, debug=False)
    # x arrives host-padded (HP x WP, zeros on the border) and pre-cast to the
    # matmul dtype, so each image loads as one contiguous DMA.
    x_in = nc.dram_tensor("x", [IPC, C, HP, WP], mm_dt, kind="ExternalInput")
    w_in = nc.dram_tensor("w", [C, C, KH * KW], f32, kind="ExternalInput")
    gt_in = nc.dram_tensor("gt", [T, T], f32, kind="ExternalInput")  # GT[u,t]=G[t,u]
    b_in = nc.dram_tensor("b", [C, 1], f32, kind="ExternalInput")
    y_out = nc.dram_tensor("y", [IPC, C, H, W_SP], f32, kind="ExternalOutput")

    with tile.TileContext(nc) as tc:
        with tc.tile_pool(name="consts", bufs=1) as cpool:
            w_sb = cpool.tile([C, C, KH * KW], f32)
            gt_sb = cpool.tile([T, T], f32)
            b_sb = cpool.tile([C, 1], f32)
            wk_sb = cpool.tile([C, KH * KW, T], mm_dt)  # fused conv weights (lhsT)
            bias_sb = cpool.tile([T, 1], f32)  # G @ b
            nc.sync.dma_start(w_sb[:], w_in[:])
            nc.sync.dma_start(gt_sb[:], gt_in[:])
            nc.sync.dma_start(b_sb[:], b_in[:])

            # Fuse the time-mix into the conv weights:
            #   wk_sb[c, k, t] = sum_u W[u, c, k] * G[t, u]
            # which is exactly the lhsT the conv matmuls need.
            with tc.tile_pool(name="fpsum", bufs=2, space="PSUM") as fpsum:
                for kk in range(KH * KW):
                    ps = fpsum.tile([C, T], f32)
                    nc.tensor.matmul(
                        ps[:], w_sb[:, :, kk], gt_sb[:], start=True, stop=True
                    )
                    nc.vector.tensor_copy(wk_sb[:, kk, :], ps[:])
                psb = fpsum.tile([T, 1], f32, tag="psb")
                nc.tensor.matmul(psb[:], gt_sb[:], b_sb[:], start=True, stop=True)
                nc.vector.tensor_copy(bias_sb[:], psb[:])

            with (
                tc.tile_pool(name="pad", bufs=2) as pad_pool,
                tc.tile_pool(name="outs", bufs=3) as out_pool,
                tc.tile_pool(name="cpsum", bufs=6, space="PSUM") as cpsum,
            ):
                grows = GROUP * ROWS  # 16 output rows per DMA batch

                def conv_body(_iv=None):
                    for i in range(IPC):
                        pb = pad_pool.tile([C, HP, WP], mm_dt, tag="pad")
                        nc.sync.dma_start(
                            pb[:, :DMA_SPLIT, :], x_in[i, :, :DMA_SPLIT, :]
                        )
                        nc.sync.dma_start(
                            pb[:, DMA_SPLIT:, :], x_in[i, :, DMA_SPLIT:, :]
                        )
                        for g0 in range(0, H, grows):
                            ot = out_pool.tile([C, grows, W_SP], f32, tag="ot")
                            for c in range(GROUP):
                                hh = g0 + c * ROWS
                                ps = cpsum.tile(
                                    [C, ROWS, W_SP], f32, name="cps", tag="cps"
                                )
                                for kk in range(KH * KW):
                                    dh, dw = divmod(kk, KW)
                                    rhs = pb[
                                        :, hh + dh : hh + dh + ROWS, dw : dw + W_SP
                                    ]
                                    nc.tensor.matmul(
                                        ps[:],
                                        wk_sb[:, kk, :],
                                        rhs,
                                        start=(kk == 0),
                                        stop=(kk == KH * KW - 1),
                                    )
                                nc.vector.tensor_scalar_add(
                                    ot[:, c * ROWS : (c + 1) * ROWS, :],
                                    ps[:],
                                    bias_sb[:],
                                )
                            nc.sync.dma_start(
                                y_out[i, :, g0 : g0 + grows, :], ot[:]
                            )

                if repeat == 1:
                    conv_body()
                else:
                    with tc.For_i(0, repeat, 1) as iv:
                        conv_body(iv)

    nc.compile()
    _BUILD_CACHE[key] = nc
    return nc


def _prep_inputs(x: np.ndarray, W: np.ndarray, b: np.ndarray, mm_dtype: str):
    G = _g_matrix()
    gt = np.ascontiguousarray(G.T)
    w3 = np.ascontiguousarray(W.reshape(C, C, KH * KW).astype(np.float32))
    b2 = np.ascontiguousarray(b.reshape(C, 1).astype(np.float32))
    np_dt = _np_mm_dtype(mm_dtype)
    xp = np.zeros((x.shape[0], C, HP, WP), np_dt)
    xp[:, :, 1 : H + 1, 1 : W_SP + 1] = x  # cast + zero-pad on host
    return xp, w3, gt, b2


def _run(nc, x, W, b, mm_dtype: str = MM_DTYPE, trace: bool = False):
    from concourse.bass_utils import run_bass_kernel_spmd

    xp, w3, gt, b2 = _prep_inputs(x, W, b, mm_dtype)
    core_ids = list(range(N_CORES))
    in_maps = [
        {"x": xp[c * IPC : (c + 1) * IPC], "w": w3, "gt": gt, "b": b2}
        for c in core_ids
    ]
    res = run_bass_kernel_spmd(nc, in_maps, core_ids, trace=trace)
    out = np.concatenate([res.results[c]["y"] for c in core_ids], axis=0)
    return out.reshape(B, C, H, W_SP), res


def kernel(x: np.ndarray, W: np.ndarray, b: np.ndarray) -> np.ndarray:
    nc = _build(repeat=1)
    out, _ = _run(nc, x, W, b)
    return out


if __name__ == "__main__":
    # CoreSim structural/numerical self-check on one core's worth of data.
    from concourse.bass_interp import CoreSim

    mm_dtype = sys.argv[1] if len(sys.argv) > 1 else MM_DTYPE
    rng = np.random.default_rng(0)
    x = rng.standard_normal((IPC, C, H, W_SP)).astype(np.float32)
    W = (rng.standard_normal((C, C, KH, KW)) * 0.05).astype(np.float32)
    b = np.zeros((C,), np.float32)
    G = _g_matrix()

    nc = _build(repeat=1, mm_dtype=mm_dtype)
    xp, w3, gt, b2 = _prep_inputs(x, W, b, mm_dtype)
    sim = CoreSim(nc)
    sim.tensor("x")[:] = xp[:IPC]
    sim.tensor("w")[:] = w3
    sim.tensor("gt")[:] = gt
    sim.tensor("b")[:] = b2
    sim.simulate()
    got = np.asarray(sim.tensor("y"))

    # numpy reference: conv then G-mix, in float64
    xpad = np.pad(x.astype(np.float64), ((0, 0), (0, 0), (1, 1), (1, 1)))
    conv = np.zeros((IPC, C, H, W_SP))
    for dh in range(KH):
        for dw in range(KW):
            patch = xpad[:, :, dh : dh + H, dw : dw + W_SP]
            conv += np.einsum(
                "oc,bchw->bohw", W[:, :, dh, dw].astype(np.float64), patch
            )
    want = np.einsum("tu,buhw->bthw", G.astype(np.float64), conv)
    err = np.abs(got - want).max()
    scale = np.abs(want).max()
    print(f"CoreSim max abs err: {err:.3e}  (scale {scale:.3f}, rel {err / scale:.3e})")
